# revision 1
# baseline (speedup 1.0000x reference)
"""kNN-accuracy (retrieval_knn) Trainium2 Bass kernel — 8-core SPMD, vocab-sharded.

Problem: acc = masked mean over n of [target[n] in top-K nearest word_vectors
to logits[n]] with N=4096, D=512, V=50000, K=10.

Algorithm (no top-k needed):
  target is in the top-K  <=>  #{v: d2[n,v] < d2[n,target[n]]} < K.
  x^2 cancels in the comparison, so rank by s[n,v] = w2[v] - 2*x_n.w_v.

Per core (vocab shard of width VS = 6272, all N rows):
  - s tiles [128n x 448v] via 5 accumulating fp32r matmuls into PSUM
    (4 contraction chunks of -2*x^T against w^T, plus a padded K=128 matmul
    adding w2[v] through a ones row).
  - Threshold T[n] = s[n, target[n]] extracted via a small gathered matmul
    that reuses the same stationary tiles, chunk order and PSUM accumulation
    order, so T[n] is bit-identical to the main pass's s[n, target[n]] and
    the self-comparison never counts (exact strict-< semantics at the
    boundary).
  - One fused DVE tensor_scalar(is_lt, scalar=T, accum_out) per tile counts
    closer words.
  - AllReduce(add) of per-core counts, then a replicated on-device finale:
    hit = count < K, acc = sum(mask*hit) / sum(mask).

All matmul operands are pre-rounded to fp32r (bf16 hi+lo) on the host and
DMA'd straight into fp32r SBUF tiles; fp32r runs the PE at full bf16 rate.
"""
import sys

for _p in ("/opt/trn_rl_repo", "/root/.axon_site/_ro/trn_rl_repo"):
    if _p not in sys.path:
        sys.path.insert(0, _p)

import numpy as np
import ml_dtypes
import concourse.mybir as mybir
import concourse.tile as tile
from concourse import bacc
from contextlib import ExitStack

N = 4096
D = 512
V = 50000
K = 10
NUM_CORES = 8
VT = 448                 # matmul moving free dim (v-tile width)
TPC = 14                 # v-tiles per core
VS = VT * TPC            # 6272 vocab columns per core
VPAD = VS * NUM_CORES    # 50176
NT = N // 128            # 32 n-tiles
DC = D // 128            # 4 contraction chunks

F32 = mybir.dt.float32
F32R = mybir.dt.float32r


def _round_fp32r(x):
    hi = x.astype(ml_dtypes.bfloat16).astype(np.float32)
    lo = (x - hi).astype(ml_dtypes.bfloat16).astype(np.float32)
    return hi + lo


def host_prep(logits, target, mask, word_vectors):
    """Shard/stage the full inputs into one input map per core."""
    logits = np.asarray(logits, dtype=np.float32)
    target = np.asarray(target).astype(np.int64)
    mask = np.asarray(mask)
    W = np.asarray(word_vectors, dtype=np.float32)

    # pad vocab with zero vectors of huge norm so they never count
    Wp = np.zeros((VPAD, D), dtype=np.float32)
    Wp[:V] = W
    w2 = np.zeros((VPAD,), dtype=np.float32)
    w2[:V] = (W.astype(np.float64) ** 2).sum(axis=1).astype(np.float32)
    w2[V:] = 1e30

    WpT_r = _round_fp32r(np.ascontiguousarray(Wp.T))             # [D, VPAD]
    w2_r = _round_fp32r(w2)
    xT_r = _round_fp32r(np.ascontiguousarray((-2.0 * logits).T))  # [D, N]

    def chunked(a, cols):
        return np.ascontiguousarray(a.reshape(DC, 128, cols).transpose(1, 0, 2))

    xT = chunked(xT_r, N)
    wgT = np.ascontiguousarray(
        WpT_r[:, target].reshape(DC, 128, N).transpose(1, 0, 2))
    w2g = np.zeros((128, N), dtype=np.float32)
    w2g[0, :] = w2_r[target]

    maskt = np.ascontiguousarray(mask.astype(np.float32).reshape(NT, 128).T)
    idm = np.eye(128, dtype=np.float32)
    ones1 = np.zeros((128, 128), dtype=np.float32)
    ones1[0, :] = 1.0

    common = dict(xT=xT, wgT=wgT, w2g=w2g, maskt=maskt, idm=idm, ones1=ones1)
    in_maps = []
    for c in range(NUM_CORES):
        sl = slice(c * VS, (c + 1) * VS)
        m = dict(common)
        m["wT"] = chunked(np.ascontiguousarray(WpT_r[:, sl]), VS)
        w2c = np.zeros((128, VS), dtype=np.float32)
        w2c[0, :] = w2_r[sl]
        m["w2c"] = w2c
        in_maps.append(m)
    return in_maps


def build_nc(num_cores=NUM_CORES):
    nc = bacc.Bacc("TRN2", target_bir_lowering=False, debug=False,
                   num_devices=num_cores)
    ins = {
        "xT": nc.dram_tensor("xT", [128, DC, N], F32, kind="ExternalInput").ap(),
        "wT": nc.dram_tensor("wT", [128, DC, VS], F32, kind="ExternalInput").ap(),
        "w2c": nc.dram_tensor("w2c", [128, VS], F32, kind="ExternalInput").ap(),
        "wgT": nc.dram_tensor("wgT", [128, DC, N], F32, kind="ExternalInput").ap(),
        "w2g": nc.dram_tensor("w2g", [128, N], F32, kind="ExternalInput").ap(),
        "maskt": nc.dram_tensor("maskt", [128, NT], F32, kind="ExternalInput").ap(),
        "idm": nc.dram_tensor("idm", [128, 128], F32, kind="ExternalInput").ap(),
        "ones1": nc.dram_tensor("ones1", [128, 128], F32, kind="ExternalInput").ap(),
    }
    acc_out = nc.dram_tensor("acc", [1, 1], F32, kind="ExternalOutput").ap()
    cnt_dbg = nc.dram_tensor("cnt_dbg", [128, NT], F32, kind="ExternalOutput").ap()

    with tile.TileContext(nc, trace_sim=False) as tc:
        _knn_kernel(tc, acc_out, cnt_dbg, ins, num_cores)
    nc.compile()
    return nc


def _knn_kernel(tc, acc_out, cnt_dbg, ins, num_cores):
    nc = tc.nc
    ctx = ExitStack()
    with ctx:
        const = ctx.enter_context(tc.tile_pool(name="const", bufs=1))
        wstream = ctx.enter_context(tc.tile_pool(name="wstream", bufs=3))
        gstream = ctx.enter_context(tc.tile_pool(name="gstream", bufs=2))
        scratch = ctx.enter_context(tc.tile_pool(name="scratch", bufs=4))
        psm = ctx.enter_context(tc.tile_pool(name="psm", bufs=5, space="PSUM"))
        psg = ctx.enter_context(tc.tile_pool(name="psg", bufs=2, space="PSUM"))
        psf = ctx.enter_context(tc.tile_pool(name="psf", bufs=1, space="PSUM"))
        dram = ctx.enter_context(tc.tile_pool(name="dram", bufs=1, space="DRAM"))

        # resident inputs (fp32r tiles take pre-rounded bits directly)
        xT_r = const.tile([128, DC, N], F32R)
        nc.sync.dma_start(xT_r[:], ins["xT"].bitcast(F32R))
        idm_t = const.tile([128, 128], F32)
        nc.sync.dma_start(idm_t[:], ins["idm"][:])
        ones_r = const.tile([128, 128], F32R)
        nc.sync.dma_start(ones_r[:], ins["ones1"].bitcast(F32R))
        mask_t = const.tile([128, NT], F32)
        nc.sync.dma_start(mask_t[:], ins["maskt"][:])
        w2g_r = const.tile([128, N], F32R)
        nc.sync.dma_start(w2g_r[:], ins["w2g"].bitcast(F32R))
        w2c_r = const.tile([128, VS], F32R)
        nc.sync.dma_start(w2c_r[:], ins["w2c"].bitcast(F32R))

        T_sb = const.tile([128, NT], F32)          # per-row thresholds
        counts = const.tile([128, NT, TPC], F32)   # per (n_tile, v_tile) counts

        # phase 1: threshold extraction via gathered matmul + diag mask
        for i in range(NT):
            wg_r = gstream.tile([128, DC, 128], F32R, tag="wg_r")
            nc.sync.dma_start(
                wg_r[:], ins["wgT"].bitcast(F32R)[:, :, i * 128:(i + 1) * 128])

            pg = psg.tile([128, 128], F32)
            for d in range(DC):
                nc.tensor.matmul(pg[:], xT_r[:, d, i * 128:(i + 1) * 128],
                                 wg_r[:, d, :], start=(d == 0), stop=False)
            nc.tensor.matmul(pg[:], ones_r[:], w2g_r[:, i * 128:(i + 1) * 128],
                             start=False, stop=True)

            scr = scratch.tile([128, 128], F32, tag="scr")
            nc.vector.tensor_tensor(scr[:], pg[:], idm_t[:],
                                    op=mybir.AluOpType.mult)
            nc.vector.tensor_reduce(T_sb[:, i:i + 1], scr[:],
                                    axis=mybir.AxisListType.X,
                                    op=mybir.AluOpType.add)

        # phase 2: main pass over this core's vocab shard
        for v in range(TPC):
            wv_r = wstream.tile([128, DC, VT], F32R, tag="wv_r")
            nc.sync.dma_start(
                wv_r[:], ins["wT"].bitcast(F32R)[:, :, v * VT:(v + 1) * VT])

            for i in range(NT):
                pm = psm.tile([128, VT], F32)
                for d in range(DC):
                    nc.tensor.matmul(pm[:], xT_r[:, d, i * 128:(i + 1) * 128],
                                     wv_r[:, d, :], start=(d == 0), stop=False)
                nc.tensor.matmul(pm[:], ones_r[:], w2c_r[:, v * VT:(v + 1) * VT],
                                 start=False, stop=True)

                cmp = scratch.tile([128, VT], F32, tag="cmp")
                nc.vector.tensor_scalar(
                    cmp[:], pm[:], T_sb[:, i:i + 1], None,
                    op0=mybir.AluOpType.is_lt,
                    op1=mybir.AluOpType.add,
                    accum_out=counts[:, i, v:v + 1],
                )

        # phase 3: reduce counts, AllReduce across cores, replicated finale
        cnt_core = const.tile([128, NT], F32)
        nc.vector.tensor_reduce(cnt_core[:], counts[:],
                                axis=mybir.AxisListType.X,
                                op=mybir.AluOpType.add)

        cnt_in = dram.tile([128, NT], F32)
        cnt_out = dram.tile([128, NT], F32, addr_space="Shared")
        nc.sync.dma_start(cnt_in[:], cnt_core[:])
        nc.gpsimd.collective_compute(
            "AllReduce", mybir.AluOpType.add,
            replica_groups=[list(range(num_cores))],
            ins=[cnt_in.opt()], outs=[cnt_out.opt()],
        )
        cnt_g = const.tile([128, NT], F32)
        nc.sync.dma_start(cnt_g[:], cnt_out[:])
        nc.sync.dma_start(cnt_dbg[:], cnt_g[:])

        hit = const.tile([128, NT], F32)
        nc.vector.tensor_scalar(hit[:], cnt_g[:], float(K) - 0.5, None,
                                op0=mybir.AluOpType.is_lt)
        mh = const.tile([128, NT], F32)
        nc.vector.tensor_tensor(mh[:], hit[:], mask_t[:],
                                op=mybir.AluOpType.mult)

        nd_f = const.tile([128, 2], F32)
        nc.vector.tensor_reduce(nd_f[:, 0:1], mh[:], axis=mybir.AxisListType.X,
                                op=mybir.AluOpType.add)
        nc.vector.tensor_reduce(nd_f[:, 1:2], mask_t[:],
                                axis=mybir.AxisListType.X,
                                op=mybir.AluOpType.add)
        nd_r = const.tile([128, 2], F32R)
        nc.vector.tensor_copy(nd_r[:], nd_f[:])
        onesc_f = const.tile([128, 1], F32)
        nc.gpsimd.memset(onesc_f[:], 1.0)
        onesc_r = const.tile([128, 1], F32R)
        nc.vector.tensor_copy(onesc_r[:], onesc_f[:])

        pf = psf.tile([1, 2], F32)
        nc.tensor.matmul(pf[:], onesc_r[:], nd_r[:], start=True, stop=True)

        nd_sb = const.tile([1, 2], F32)
        nc.vector.tensor_copy(nd_sb[:], pf[:])
        rec_t = const.tile([1, 1], F32)
        nc.vector.reciprocal(rec_t[:], nd_sb[:, 1:2])
        acc_t = const.tile([1, 1], F32)
        nc.vector.tensor_tensor(acc_t[:], nd_sb[:, 0:1], rec_t[:],
                                op=mybir.AluOpType.mult)
        nc.sync.dma_start(acc_out[:], acc_t[:])


_NC_CACHE = {}


def _get_nc():
    if "nc" not in _NC_CACHE:
        _NC_CACHE["nc"] = build_nc()
    return _NC_CACHE["nc"]


def kernel(logits, target, mask, word_vectors):
    """Full inputs in, full output out (shape [1] float32)."""
    from concourse.bass_utils import run_bass_kernel_spmd

    in_maps = host_prep(logits, target, mask, word_vectors)
    nc = _get_nc()

    last_err = None
    for attempt in range(3):
        try:
            res = run_bass_kernel_spmd(nc, in_maps, list(range(NUM_CORES)))
            acc = np.asarray(res.results[0]["acc"]).reshape(1).astype(np.float32)
            return acc
        except Exception as e:  # transient NRT/axon failures: retry
            last_err = e
    raise last_err
